# revision 1
# baseline (speedup 1.0000x reference)
"""KMeans-LSE kernel for Trainium2 (8 NeuronCores, data-parallel over N).

Computes, for x (65536, 256) f32 and centroids (1024, 256) f32:
    sq[n,k] = ||x_n - c_k||^2
    y[n]    = lse(beta*sq[n,:], axis=k) / beta     with beta = -1
i.e.  y[n] = minsq[n] - log(sum_k exp(minsq[n] - sq[n,k]))

Strategy (per core, N_loc = 8192 rows):
  - u[n,k] = c2[k] - 2*dot[n,k] is accumulated fully in PSUM by the
    TensorE: 4 f32r matmuls for -2*x@C.T plus 2 contract-1 matmuls that
    broadcast-add c2[k] (ones-column lhsT trick).
  - qm[n] = min_k u[n,k] via one VectorE tensor_reduce straight from PSUM.
    (qm = minsq - x2, exact.)
  - S[n] = sum_k exp(qm - u) via one ScalarE activation (Exp with
    per-partition bias=qm, scale=-1) with fused accum_out.
  - x2[n] = sum_d x[n,d]^2 on GpSimd (square + add-tree), finished in the
    epilogue.
  - y = x2 + qm - log(S), computed once for all 64 blocks at the end.
  x is transposed on-chip (TensorE transpose via identity, PSUM->SBUF
  copies split between VectorE and ScalarE).
"""

import numpy as np

_CACHE = {}

N, D, K = 65536, 256, 1024
NCORES = 8
NLOC = N // NCORES          # 8192 rows per core
P = 128
BLOCKS = NLOC // P          # 64 blocks of 128 rows
QS = 4                      # blocks per DMA super-load
SUPERS = BLOCKS // QS       # 16
TREE_OUT = 16               # x2 partial width left for the epilogue reduce


def _build(matmul_dtype_name="float32r"):
    import concourse.mybir as mybir
    import concourse.tile as tile
    from concourse import bacc
    from concourse.masks import make_identity

    f32 = mybir.dt.float32
    mmdt = getattr(mybir.dt, matmul_dtype_name)
    AF = mybir.ActivationFunctionType
    ALU = mybir.AluOpType

    nc = bacc.Bacc(
        "TRN2",
        target_bir_lowering=False,
        debug=False,
        enable_asserts=False,
        num_devices=NCORES,
    )
    xs = nc.dram_tensor("xs", [NLOC, D], f32, kind="ExternalInput").ap()
    cent = nc.dram_tensor("cent", [K, D], f32, kind="ExternalInput").ap()
    y = nc.dram_tensor("y", [NLOC], f32, kind="ExternalOutput").ap()

    with tile.TileContext(nc) as tc:
        with (
            tc.tile_pool(name="res", bufs=1) as res,
            tc.tile_pool(name="setup", bufs=1) as setupp,
            tc.tile_pool(name="xp", bufs=3) as xp,
            tc.tile_pool(name="xtp", bufs=3) as xtp,
            tc.tile_pool(name="ejp", bufs=1) as ejp,
            tc.tile_pool(name="sqp", bufs=3) as sqp,
            tc.tile_pool(name="ups", bufs=3, space="PSUM") as ups,
            tc.tile_pool(name="mps", bufs=2, space="PSUM") as mps,
        ):
            # ---------------- residents ----------------
            ident = res.tile([P, P], f32)
            make_identity(nc, ident)
            onesc = res.tile([P, 1], f32)
            nc.vector.memset(onesc, 1.0)
            ones1f = res.tile([1, P], f32)
            nc.vector.memset(ones1f, 1.0)
            ones1 = res.tile([1, P], mmdt)
            nc.vector.tensor_copy(ones1, ones1f)
            CsTs = res.tile([P, 2, K], mmdt)    # -2 * centroids^T
            c2row = res.tile([1, K], mmdt)      # sum(c^2) per centroid
            qm_all = res.tile([P, BLOCKS], f32)
            S_all = res.tile([P, BLOCKS], f32)
            x2p_all = res.tile([P, BLOCKS, TREE_OUT], f32)

            # ---------------- setup: centroid prep ----------------
            ct = setupp.tile([P, K // P, D], f32)
            nc.sync.dma_start(ct, cent.rearrange("(t p) d -> p t d", p=P))
            # transpose C -> CsTs (raw for now), 16 PE transposes
            for t in range(K // P):
                for c in range(2):
                    tp = mps.tile([P, 2, P], f32, tag="xT_ps")
                    nc.tensor.transpose(
                        tp[:, 0, :], ct[:, t, c * P:(c + 1) * P], ident
                    )
                    dst = CsTs[:, c, t * P:(t + 1) * P]
                    if (t + c) % 2 == 0:
                        nc.vector.tensor_copy(dst, tp[:, 0, :])
                    else:
                        nc.scalar.copy(dst, tp[:, 0, :])
            # c2row = colsum over d of CsTs^2 (before the -2 scaling)
            csq = setupp.tile([P, 2, K], f32)
            nc.scalar.activation(csq, CsTs, AF.Square)
            c2ps = ups.tile([P, K], f32, tag="u")
            for ks in range(2):
                for c in range(2):
                    nc.tensor.matmul(
                        c2ps[0:1, ks * 512:(ks + 1) * 512],
                        lhsT=onesc,
                        rhs=csq[:, c, ks * 512:(ks + 1) * 512],
                        start=(c == 0),
                        stop=(c == 1),
                    )
            nc.vector.tensor_copy(c2row, c2ps[0:1, :])
            # scale centroids by -2 (after c2 extraction)
            nc.vector.tensor_scalar_mul(CsTs, CsTs, -2.0)

            # ---------------- main loop ----------------
            xs_r = xs.rearrange("(s q p) d -> s p q d", p=P, q=QS)
            for s in range(SUPERS):
                x_sb = xp.tile([P, QS, D], f32, tag="x")
                nc.sync.dma_start(x_sb, xs_r[s])
                for q in range(QS):
                    j = s * QS + q
                    xq = x_sb[:, q, :]
                    # transpose x block -> xT  (PSUM then SBUF)
                    xT_ps = mps.tile([P, 2, P], f32, tag="xT_ps")
                    nc.tensor.transpose(xT_ps[:, 0, :], xq[:, 0:P], ident)
                    nc.tensor.transpose(xT_ps[:, 1, :], xq[:, P:D], ident)
                    xT = xtp.tile([P, 2, P], mmdt, tag="xT")
                    nc.vector.tensor_copy(xT[:, 0, :], xT_ps[:, 0, :])
                    nc.scalar.copy(xT[:, 1, :], xT_ps[:, 1, :])
                    # u = c2 - 2 x@C.T  accumulated in PSUM
                    u = ups.tile([P, K], f32, tag="u")
                    for ks in range(2):
                        sl = slice(ks * 512, (ks + 1) * 512)
                        nc.tensor.matmul(
                            u[:, sl],
                            lhsT=xT[:, 0, :],
                            rhs=CsTs[:, 0, sl],
                            start=True,
                            stop=False,
                        )
                        nc.tensor.matmul(
                            u[:, sl],
                            lhsT=xT[:, 1, :],
                            rhs=CsTs[:, 1, sl],
                            start=False,
                            stop=False,
                        )
                        nc.tensor.matmul(
                            u[:, sl],
                            lhsT=ones1,
                            rhs=c2row[:, sl],
                            start=False,
                            stop=True,
                        )
                    # qm = min_k u   (straight from PSUM)
                    nc.vector.tensor_reduce(
                        out=qm_all[:, j:j + 1],
                        in_=u,
                        axis=mybir.AxisListType.X,
                        op=ALU.min,
                    )
                    # S = sum_k exp(qm - u)
                    ej = ejp.tile([P, K], f32, tag="ej")
                    nc.scalar.activation(
                        ej,
                        u,
                        AF.Exp,
                        bias=qm_all[:, j:j + 1],
                        scale=-1.0,
                        accum_out=S_all[:, j:j + 1],
                    )
                    # x2 partials on GpSimd: square + add-tree down to 16
                    xsq = sqp.tile([P, D], f32, tag="xsq")
                    nc.gpsimd.tensor_mul(xsq, xq, xq)
                    w = D // 2
                    while w > TREE_OUT:
                        nc.gpsimd.tensor_add(
                            xsq[:, 0:w], xsq[:, 0:w], xsq[:, w:2 * w]
                        )
                        w //= 2
                    nc.gpsimd.tensor_add(
                        x2p_all[:, j, :], xsq[:, 0:TREE_OUT],
                        xsq[:, TREE_OUT:2 * TREE_OUT],
                    )

            # ---------------- epilogue ----------------
            x2_all = res.tile([P, BLOCKS], f32)
            nc.vector.tensor_reduce(
                out=x2_all,
                in_=x2p_all,
                axis=mybir.AxisListType.X,
                op=ALU.add,
            )
            logS = res.tile([P, BLOCKS], f32)
            nc.scalar.activation(logS, S_all, AF.Ln)
            outv = res.tile([P, BLOCKS], f32)
            nc.vector.tensor_add(outv, x2_all, qm_all)
            nc.vector.tensor_sub(outv, outv, logS)
            # transpose [128, 64] -> [64, 128] so the store is contiguous
            out_ps = mps.tile([P, 2, P], f32, tag="xT_ps")
            nc.tensor.transpose(out_ps[0:BLOCKS, 0, :], outv, ident)
            outT = res.tile([BLOCKS, P], f32)
            nc.vector.tensor_copy(outT, out_ps[0:BLOCKS, 0, :])
            nc.sync.dma_start(y.rearrange("(j p) -> j p", p=P), outT)

    nc.compile()
    return nc


def _get_nc():
    key = "nc"
    if key not in _CACHE:
        _CACHE[key] = _build()
    return _CACHE[key]


def kernel(x, centroids):
    from concourse import bass_utils

    x = np.ascontiguousarray(np.asarray(x, dtype=np.float32))
    centroids = np.ascontiguousarray(np.asarray(centroids, dtype=np.float32))
    assert x.shape == (N, D) and centroids.shape == (K, D)

    nc = _get_nc()
    in_maps = [
        {"xs": x[i * NLOC:(i + 1) * NLOC], "cent": centroids}
        for i in range(NCORES)
    ]
    res = bass_utils.run_bass_kernel_spmd(
        nc, in_maps, core_ids=list(range(NCORES))
    )
    return np.concatenate([res.results[i]["y"] for i in range(NCORES)])

